# revision 37
# baseline (speedup 1.0000x reference)
"""3-layer GCN on 8 Trainium2 NeuronCores — v3 (col-packed fp8).

Key ideas vs v2 (406us):
  - All matmuls drop DoubleRow and instead use tile_position column
    packing: out free dim F (64/32/16) < 128, so 2 (F=64) or 4 (F<=32)
    matmuls run CONCURRENTLY in disjoint 32/64-column groups of the PE
    array (measured 55.4 ns per [32,512] mm, 4.2x vs DR's 231 ns).
    Each PSUM bank [128,512] holds G group accumulators; a DVE add
    tree sums them in the epilogue.
  - Bit-plane expansion fused: one tensor_scalar per (hs, d, k) covers
    both gg groups and both target halves ([128, 2, 2048]B, 687 ns).
  - The per-source dis scale in wmul moves from DVE to the ACT engine
    (activation scale AP), freeing DVE for expansion.
  - Layers iterate hs-outer: srcA (hs=0) matmuls only need the first
    AllGather half, so they fill the AG-B latency window; L1 srcA
    blocks interleave into the back half of stage-1 h1.

Layout conventions (per core d; N=16384, CP=2048):
  local node n = p*16 + m,  m = h*8 + mm          (p partition, h half)
  column order in all [F, 2048] feature tiles:  col' = h*1024 + p*8 + mm
  src plane (d, h, mm): srcs {d*2048 + p*16 + h*8 + mm : p in 0..127}
  packed A byte [p, g, t'], g = d*4 + h*2 + gg, bit k:  plane mm = 4gg+k
  psf[li,h][p, d*8 + mm, f] = feature f of src (d, h, p, mm)
  AllGather row (per half): r = d*1024 + p*8 + mm.
"""

import numpy as np

import concourse.bass as bass
import concourse.mybir as mybir
import concourse.tile as tile
from concourse import bacc
from concourse.bass_utils import run_bass_kernel_spmd
from concourse.masks import make_identity

N = 16384
NCORES = 8
CP = N // NCORES          # 2048
F1, F2, F3 = 64, 32, 16
KT = N // 128             # 128 k tiles
MT = 16

F32 = mybir.dt.float32
FP8 = mybir.dt.float8e4
I32 = mybir.dt.int32
U8 = mybir.dt.uint8
BF16 = mybir.dt.bfloat16
NP_FP8 = mybir.dt.np(FP8)
NP_BF16 = mybir.dt.np(BF16)

_prog_cache = {}


def _build_program():
    nc = bacc.Bacc("TRN2", target_bir_lowering=False, debug=False,
                   num_devices=NCORES)

    # x^T fp8, pre-tiled: [p, (h, kt, t)] = x8[node n(h,t), kt*128+p]
    xt_d = nc.dram_tensor("xt_d", [128, 2 * KT * 1024], FP8,
                          kind="ExternalInput")
    # packed adjacency bit planes [p, (g, t')]
    a_d = nc.dram_tensor("a_d", [128, 32 * CP], U8, kind="ExternalInput")
    # pre-expanded dense fp8 planes for DENSE_BLOCKS (DMA-offloaded
    # expansion: rides the post-stage-1 idle DMA instead of the DVE)
    adense_d = nc.dram_tensor("adense_d", [128, 4 * 8 * CP], U8,
                              kind="ExternalInput")
    w1_d = nc.dram_tensor("w1_d", [128, KT * F1], FP8, kind="ExternalInput")
    w2_d = nc.dram_tensor("w2_d", [F1, F2], F32, kind="ExternalInput")
    w3_d = nc.dram_tensor("w3_d", [F2, F3], F32, kind="ExternalInput")
    b1_d = nc.dram_tensor("b1_d", [F1, 1], F32, kind="ExternalInput")
    b2_d = nc.dram_tensor("b2_d", [F2, 1], F32, kind="ExternalInput")
    b3_d = nc.dram_tensor("b3_d", [F3, 1], F32, kind="ExternalInput")
    dis16_d = nc.dram_tensor("dis16_d", [128, MT], F32, kind="ExternalInput")
    disrep_d = nc.dram_tensor("disrep_d", [F1, CP], BF16, kind="ExternalInput")
    ridxa_d = nc.dram_tensor("ridxa_d", [128, 1], I32, kind="ExternalInput")
    ridxb_d = nc.dram_tensor("ridxb_d", [128, 1], I32, kind="ExternalInput")
    rcols_d = nc.dram_tensor("rcols_d", [128, CP], FP8, kind="ExternalInput")
    out_d = nc.dram_tensor("out_d", [CP, F3], F32, kind="ExternalOutput")

    # AllGather bounce buffers per (layer, half)
    ps_in = {}
    ps_out = {}
    for li, F in ((1, F1), (2, F2), (3, F3)):
        for h in (0, 1):
            ps_in[li, h] = nc.dram_tensor(f"ps_in_{li}_{h}", [1024, F], FP8)
            ps_out[li, h] = nc.dram_tensor(f"ps_out_{li}_{h}", [8192, F],
                                           FP8, addr_space="Shared")

    with tile.TileContext(nc) as tc:
        with tc.tile_pool(name="const", bufs=1) as cpool, \
             tc.tile_pool(name="xt", bufs=6) as xtpool, \
             tc.tile_pool(name="exp", bufs=3) as epool, \
             tc.tile_pool(name="apk", bufs=4) as apool, \
             tc.tile_pool(name="work", bufs=2) as wpool, \
             tc.tile_pool(name="big", bufs=1) as bpool, \
             tc.tile_pool(name="psum", bufs=2, space="PSUM") as psum, \
             tc.tile_pool(name="psum_acc", bufs=1, space="PSUM") as psum_acc:

            # ---- constants -------------------------------------------------
            # w1 rides first on the scalar ring (needed at stage-1 kb0);
            # everything else goes on the (early-idle) gpsimd ring so the
            # x stream is never queued behind const traffic.
            w1_sb = cpool.tile([128, KT * F1], FP8, tag="w1")
            nc.scalar.dma_start(out=w1_sb[:], in_=w1_d[:, :])
            w2_sb = cpool.tile([F1, F2], F32, tag="w2")
            nc.gpsimd.dma_start(out=w2_sb[:], in_=w2_d[:, :])
            w3_sb = cpool.tile([F2, F3], F32, tag="w3")
            nc.gpsimd.dma_start(out=w3_sb[:], in_=w3_d[:, :])
            b_sb = {}
            for li, (bd, F) in ((1, (b1_d, F1)), (2, (b2_d, F2)),
                                (3, (b3_d, F3))):
                b_sb[li] = cpool.tile([F, 1], F32, tag=f"b{li}",
                                      name=f"b{li}_sb")
                nc.gpsimd.dma_start(out=b_sb[li][:], in_=bd[:, :])
            dis16_sb = cpool.tile([128, MT], F32, tag="dis16")
            nc.gpsimd.dma_start(out=dis16_sb[:], in_=dis16_d[:, :])
            disrep_sb = cpool.tile([F1, CP], BF16, tag="disrep")
            nc.gpsimd.dma_start(out=disrep_sb[:], in_=disrep_d[:, :])
            ridxa_sb = cpool.tile([128, 1], I32, tag="ridxa")
            nc.gpsimd.dma_start(out=ridxa_sb[:], in_=ridxa_d[:, :])
            ridxb_sb = cpool.tile([128, 1], I32, tag="ridxb")
            nc.gpsimd.dma_start(out=ridxb_sb[:], in_=ridxb_d[:, :])
            rcols_sb = cpool.tile([128, CP], FP8, tag="rcols")
            nc.gpsimd.dma_start(out=rcols_sb[:], in_=rcols_d[:, :])
            ident = cpool.tile([128, 128], F32, tag="ident")
            make_identity(nc, ident[:])

            # ---- PE warm-up: ~5us of back-to-back matmuls during the
            # initial DMA wait gets the HAM clock gate to K=8/8 before the
            # stage-1 stream arrives (cold PE runs at 1.2 instead of 2.4
            # GHz and stalls the whole pipeline behind it).  The tile uses
            # tag acc4, which the L1 aggregation reuses later.
            warm_ps = psum_acc.tile([128, 512], F32, tag="acc4", name="warm")

            def warm_mms(n):
                for _ in range(n):
                    nc.tensor.matmul(warm_ps[0:64, 0:128],
                                     lhsT=ident[:, 0:64],
                                     rhs=ident[:, :], start=True, stop=True,
                                     skip_group_check=True)
            warm_mms(48)

            # packed adjacency is STREAMED per expansion block (8MB/layer
            # rides the post-stage-1 idle DMA; keeps stage-1 pure-x and
            # frees 48KB of SBUF for deeper xt/et pools)

            # gathered features per (layer, half): [p, d*8+mm, F] fp8
            psf = {}
            for li, F in ((1, F1), (2, F2), (3, F3)):
                for h in (0, 1):
                    psf[li, h] = cpool.tile([128, 64, F], FP8,
                                            tag=f"psf{li}{h}",
                                            name=f"psf{li}{h}")

            def strided_m(t, m):
                """[F, CP] col'-ordered tile -> [F, 128] slice of nodes with
                n%16 == m (partitions p in order)."""
                return t[:].rearrange("f (hh pp mm) -> f hh mm pp",
                                      hh=2, mm=8)[:, m // 8, m % 8, :]

            # ---- stage 1 PSUM: per (h, c) one [128,512] bank, col groups
            # by kt parity (rows 0-63 even kt, 64-127 odd kt).
            s1_ps = [psum_acc.tile([128, 512], F32, tag=f"acc{c}",
                                   name=f"s1_ps{c}") for c in range(4)]

            # ---- aggregation helpers --------------------------------------
            DENSE_BLOCKS = {(0, 2): 0, (0, 5): 1, (1, 2): 2, (1, 5): 3}

            def expand_block(li, hs, d):
                """expand the 8 planes of source block (hs, d) for ALL 2048
                targets -> et [128, k, gg, 2048] u8 (fp8 {0, 2.0})."""
                if (hs, d) in DENSE_BLOCKS:
                    s = DENSE_BLOCKS[hs, d]
                    et = epool.tile([128, 4, 2, 2048], U8, tag="exp",
                                    name=f"e{li}{hs}{d}")
                    ring = nc.sync if s % 2 == 0 else nc.scalar
                    ring.dma_start(
                        out=et[:],
                        in_=adense_d[:, s * 8 * CP:(s + 1) * 8 * CP]
                            .rearrange("p (k g t) -> p k g t", k=4, g=2))
                    return et
                g0 = 4 * d + 2 * hs
                asl = apool.tile([128, 2, 2048], U8, tag="apack",
                                 name=f"a{li}{hs}{d}")
                aring = nc.sync if (hs * 8 + d) % 2 == 0 else nc.scalar
                aring.dma_start(
                    out=asl[:],
                    in_=a_d[:, g0 * CP:(g0 + 2) * CP]
                        .rearrange("p (g t) -> p g t", g=2))
                et = epool.tile([128, 4, 2, 2048], U8, tag="exp",
                                name=f"e{li}{hs}{d}")
                # Mild deprioritization (about one layer's instruction
                # span): keeps each layer's pool-paced expansions BEHIND
                # that layer's latency-critical epilogue ops in the DVE
                # order, without reordering across layers (a global
                # deprioritization was tried and created worse couplings).
                with tc.high_priority(offset=-1500):
                    for k in range(4):
                        nc.vector.tensor_scalar(
                            out=et[:, k, :, :].bitcast(I32),
                            in0=asl[:].bitcast(I32),
                            scalar1=0x01010101 << k, scalar2=6 - k,
                            op0=mybir.AluOpType.bitwise_and,
                            op1=mybir.AluOpType.logical_shift_left)
                return et

            def agg_block_mms(li, F, G, hs, d, banks, started, last):
                """col-packed matmuls consuming one expanded block.  F<=32
                packs 4-way by plane k; F==64 feature-splits each plane so
                it also packs 4-way (cold-clock robust)."""
                et = expand_block(li, hs, d)
                fsplit = 2 if F == 64 else 1
                Fs = F // fsplit
                for gg in range(2):
                    for cc in range(4):
                        for k in range(4):
                            lhsT = psf[li, hs][:, d * 8 + 4 * gg + k, :]
                            rhs = et[:, k, gg, cc * 512:(cc + 1) * 512] \
                                .bitcast(FP8)
                            for fh in range(fsplit):
                                if fsplit == 2:
                                    g = 2 * (k % 2) + fh
                                    lh = lhsT[:, 32 * fh:32 * fh + 32]
                                    # groups 0,1 (k even, rows 0-63) stay
                                    # open for the resid matmuls
                                    stp = (last and gg == 1 and k == 3)
                                else:
                                    g = k
                                    lh = lhsT
                                    stp = (last and gg == 1 and k > 0)
                                cb = 32 * g
                                key = (cc, cb)
                                st = key not in started
                                started.add(key)
                                nc.tensor.matmul(
                                    banks[cc][cb:cb + Fs, :], lhsT=lh,
                                    rhs=rhs, start=st, stop=stp,
                                    tile_position=(0, cb))

            def resid_gather(li, F):
                rh = wpool.tile([128, F], FP8, tag="residh",
                                name=f"residh{li}")
                for h, ridx in ((0, ridxa_sb), (1, ridxb_sb)):
                    nc.gpsimd.indirect_dma_start(
                        out=rh[:], out_offset=None,
                        in_=ps_out[li, h].ap(),
                        in_offset=bass.IndirectOffsetOnAxis(
                            ap=ridx[:, :1], axis=0),
                        bounds_check=8191, oob_is_err=False)
                return rh

            def resid_mms(rh, F, banks):
                for cc in range(4):
                    nc.tensor.matmul(
                        banks[cc][0:F, :], lhsT=rh[:],
                        rhs=rcols_sb[:, cc * 512:(cc + 1) * 512],
                        start=False, stop=True, skip_group_check=True,
                        tile_position=(0, 0))

            def psum_group_sum(bank, F, G, zs, tagp):
                """sum the G col-group accumulators of one PSUM bank into
                zs [F, 512] (SBUF).  ACT copies the upper groups out of
                PSUM (cross-partition), DVE adds."""
                gbase = (64,) if G == 2 else (32, 64, 96)
                cp = []
                for gi, gb in enumerate(gbase):
                    c = wpool.tile([F, 512], F32, tag=f"cp{gi}",
                                   name=f"cp{tagp}_{gi}")
                    nc.scalar.activation(
                        c[:], bank[gb:gb + F, :],
                        mybir.ActivationFunctionType.Identity,
                        bias=0.0)
                    cp.append(c)
                if G == 2:
                    nc.vector.tensor_tensor(zs[:], bank[0:F, :], cp[0][:],
                                            mybir.AluOpType.add)
                else:
                    t1 = wpool.tile([F, 512], F32, tag="t1",
                                    name=f"t1{tagp}")
                    nc.vector.tensor_tensor(t1[:], bank[0:F, :], cp[0][:],
                                            mybir.AluOpType.add)
                    t2 = wpool.tile([F, 512], F32, tag="t2",
                                    name=f"t2{tagp}")
                    nc.vector.tensor_tensor(t2[:], cp[1][:], cp[2][:],
                                            mybir.AluOpType.add)
                    nc.vector.tensor_tensor(zs[:], t1[:], t2[:],
                                            mybir.AluOpType.add)

            def agg_epilogue(li, ht, F, G, banks, hT_sb):
                func = (mybir.ActivationFunctionType.Relu if li < 3
                        else mybir.ActivationFunctionType.Identity)
                for cc in (2 * ht, 2 * ht + 1):
                    sl = slice(cc * 512, (cc + 1) * 512)
                    zs = wpool.tile([F, 512], F32, tag="zs",
                                    name=f"zs{li}_{cc}")
                    psum_group_sum(banks[cc], F, G, zs, f"{li}_{cc}")
                    zt = wpool.tile([F, 512], F32, tag="zt",
                                    name=f"zt{li}_{cc}")
                    nc.vector.tensor_tensor(zt[:], zs[:],
                                            disrep_sb[:F, sl],
                                            mybir.AluOpType.mult)
                    nc.scalar.activation(hT_sb[:, sl], zt[:], func,
                                         bias=b_sb[li][:, 0:1])

            def wmul_gather(hT_sb, li, F_nxt, w_sb, h):
                """W-mul (or stage-1 transpose) + dis pre-scale + fp8,
                bounce to DRAM, AllGather half h, reload psf[li, h].
                All 8 m-slices land in ONE PSUM bank so a single DVE
                multiply (broadcast dis16) replaces 8 serial ACT ops."""
                pt = psum.tile([128, 8, F_nxt], F32, tag="wmul",
                               name=f"wm{li}_{h}")
                for mm in range(8):
                    m = h * 8 + mm
                    if w_sb is None:
                        nc.tensor.transpose(pt[:, mm, :],
                                            strided_m(hT_sb, m),
                                            ident[:F_nxt, :F_nxt])
                    else:
                        nc.tensor.matmul(pt[:, mm, :],
                                         lhsT=strided_m(hT_sb, m),
                                         rhs=w_sb[:], start=True, stop=True)
                ps_local = wpool.tile([128, 8, F_nxt], FP8,
                                      tag=f"psl{li}", name=f"psl{li}_{h}")
                d16b = dis16_sb[:, h * 8:h * 8 + 8] \
                    .rearrange("p (m o) -> p m o", o=1) \
                    .broadcast_to((128, 8, F_nxt))
                nc.vector.tensor_tensor(ps_local[:], pt[:], d16b,
                                        mybir.AluOpType.mult)
                # bounce rides gpsimd: on sync it would stall queued x
                # tiles behind its wait for ps_local
                nc.gpsimd.dma_start(
                    out=ps_in[li, h].ap().rearrange("(p mm) f -> p mm f",
                                                    p=128),
                    in_=ps_local[:])
                nc.gpsimd.collective_compute(
                    "AllGather", mybir.AluOpType.bypass,
                    replica_groups=[list(range(NCORES))],
                    ins=[ps_in[li, h].ap().opt()],
                    outs=[ps_out[li, h].ap().opt()],
                )
                nc.scalar.dma_start(
                    out=psf[li, h][:].rearrange("p (d mm) f -> p d mm f",
                                                d=8),
                    in_=ps_out[li, h].ap().rearrange("(d p mm) f -> p d mm f",
                                                     d=8, p=128))

            # ---- stage 1 ---------------------------------------------------
            s1rings = [nc.sync, nc.scalar]

            def stage1_half(h, extra_pe=None):
                xt_tile = None
                for kb in range(KT // 8):
                    xt_tile = xtpool.tile([128, 8, 1024], FP8, tag="xt",
                                          name=f"xt{h}_{kb}")
                    off = (h * KT + kb * 8) * 1024
                    s1rings[kb % 2].dma_start(
                        out=xt_tile[:],
                        in_=xt_d[:, off:off + 8 * 1024]
                            .rearrange("p (b t) -> p b t", b=8))
                    if h == 0 and kb < 6:
                        warm_mms(6)  # keep HAM at K=8/8 through DMA ramp
                    if extra_pe is not None and kb in extra_pe:
                        extra_pe[kb]()
                    for b in range(8):
                        kt = kb * 8 + b
                        kp = kt % 2
                        for c in range(2):
                            for fh in range(2):
                                g = 2 * kp + fh
                                nc.tensor.matmul(
                                    s1_ps[2 * h + c][32 * g:32 * g + 32, :],
                                    lhsT=w1_sb[:, kt * F1 + 32 * fh:
                                               kt * F1 + 32 * fh + 32],
                                    rhs=xt_tile[:, b, c * 512:(c + 1) * 512],
                                    start=(kt < 2), stop=(kt >= KT - 2),
                                    tile_position=(0, 32 * g))
                return xt_tile

            def stage1_epilogue(h, p1t_sb):
                for c in range(2):
                    sl = slice(h * 1024 + c * 512, h * 1024 + (c + 1) * 512)
                    bank = s1_ps[2 * h + c]
                    cp = wpool.tile([64, 512], F32, tag="cp0",
                                    name=f"s1cp_{h}_{c}")
                    nc.scalar.activation(
                        cp[:], bank[64:128, :],
                        mybir.ActivationFunctionType.Identity, bias=0.0)
                    nc.vector.tensor_tensor(p1t_sb[:, sl], bank[0:64, :],
                                            cp[:], mybir.AluOpType.add)
                wmul_gather(p1t_sb, 1, F1, None, h)

            # ======== stage 1 half 0 ========
            stage1_half(0)
            p1t_sb = bpool.tile([F1, CP], F32, tag="hT1")
            stage1_epilogue(0, p1t_sb)

            # L1 aggregation state
            agg1_ps = [psum_acc.tile([128, 512], F32,
                                     tag=f"acc{c if c < 2 else c + 2}",
                                     name=f"agg1_{c}")
                       for c in range(4)]
            started1 = set()

            # NOTE: no L1 blocks interleave into stage-1 h1.  NEFF launch
            # skew across cores is ~50us, so AG-A completes AFTER the
            # (DMA-bound) stage-1 stream ends; a psf-waiting matmul placed
            # inside the stage-1 PE stream would stall its tail.

            # ======== stage 1 half 1 ========
            last_xt = stage1_half(1)
            warm_mms(16)
            stage1_epilogue(1, p1t_sb)
            # PE filler across the AG-A latency window: dependency-free
            # matmuls on the last x tile keep the HAM clock gate at 8/8
            # (a >3us PE idle here re-throttles the PE to 1.2 GHz and it
            # has been observed to stay cold for ~50us afterwards).
            for i in range(64):
                nc.tensor.matmul(warm_ps[0:32, :],
                                 lhsT=w1_sb[:, 32 * (i % 8):32 * (i % 8) + 32],
                                 rhs=last_xt[:, 7, 0:512], start=True,
                                 stop=True, skip_group_check=True)

            # ======== layers ========
            specs = [(1, F1, 2, F2, w2_sb), (2, F2, 4, F3, w3_sb),
                     (3, F3, 4, None, None)]
            for li, F, G, F_nxt, w_sb in specs:
                if li == 1:
                    agg_ps, started = agg1_ps, started1
                else:
                    agg_ps = [psum_acc.tile([128, 512], F32,
                                            tag=f"acc{c if c < 2 else c + 2}",
                                            name=f"agg{li}_{c}")
                              for c in range(4)]
                    started = set()
                rh = resid_gather(li, F)
                hT_sb = bpool.tile([F, CP], F32,
                                   tag="hT1" if li == 1 else "hTb",
                                   name=f"hT{li}")
                for hs in (0, 1):
                    for d in range(8):
                        agg_block_mms(li, F, G, hs, d, agg_ps, started,
                                      last=(hs == 1 and d == 7))
                resid_mms(rh, F, agg_ps)
                for ht in (0, 1):
                    agg_epilogue(li, ht, F, G, agg_ps, hT_sb)
                    if li < 3:
                        wmul_gather(hT_sb, li + 1, F_nxt, w_sb, ht)
                    else:
                        h3 = wpool.tile([128, 8, F3], F32, tag="h3",
                                        name=f"h3_{ht}")
                        for mm in range(8):
                            m = ht * 8 + mm
                            pt = psum.tile([128, F3], F32, tag="wmul",
                                           name=f"tr3_{ht}_{mm}")
                            nc.tensor.transpose(pt[:], strided_m(hT_sb, m),
                                                ident[:F3, :F3])
                            nc.vector.tensor_copy(h3[:, mm, :], pt[:])
                        mx = wpool.tile([128, 8], F32, tag="mx",
                                        name=f"mx{ht}")
                        nc.vector.reduce_max(mx[:], h3[:],
                                             mybir.AxisListType.X,
                                             negate=True)
                        mxb = mx[:].rearrange("p (m o) -> p m o", o=1) \
                                   .broadcast_to((128, 8, F3))
                        ex = wpool.tile([128, 8, F3], F32, tag="ex",
                                        name=f"ex{ht}")
                        nc.vector.tensor_tensor(ex[:], h3[:], mxb,
                                                mybir.AluOpType.add)
                        nc.scalar.activation(ex[:], ex[:],
                                             mybir.ActivationFunctionType.Exp)
                        sm = wpool.tile([128, 8], F32, tag="sm",
                                        name=f"sm{ht}")
                        nc.vector.reduce_sum(sm[:], ex[:],
                                             mybir.AxisListType.X)
                        rc = wpool.tile([128, 8], F32, tag="rc",
                                        name=f"rc{ht}")
                        nc.vector.reciprocal(rc[:], sm[:])
                        rcb = rc[:].rearrange("p (m o) -> p m o", o=1) \
                                   .broadcast_to((128, 8, F3))
                        o_sb = wpool.tile([128, 8, F3], F32, tag="osm",
                                          name=f"osm{ht}")
                        nc.vector.tensor_tensor(o_sb[:], ex[:], rcb,
                                                mybir.AluOpType.mult)
                        nc.sync.dma_start(
                            out=out_d.ap().rearrange(
                                "(p hh mm) c -> p hh mm c",
                                p=128, hh=2)[:, ht, :, :],
                            in_=o_sb[:])

    nc.compile()
    return nc


def _get_program():
    if "nc" not in _prog_cache:
        _prog_cache["nc"] = _build_program()
    return _prog_cache["nc"]


def _preprocess(x, edge_index, W1, b1, W2, b2, W3, b3):
    x = np.asarray(x, dtype=np.float32)
    ei = np.asarray(edge_index)
    row = ei[0].astype(np.int64)
    col = ei[1].astype(np.int64)

    deg = np.bincount(col, minlength=N).astype(np.float32) + 1.0
    dis = (1.0 / np.sqrt(deg)).astype(np.float32)

    # unique edge cells incl. self loops
    pair = np.concatenate([row * N + col,
                           np.arange(N, dtype=np.int64) * (N + 1)])
    u, cnt = np.unique(pair, return_counts=True)
    us = (u // N).astype(np.int64)
    ut = (u % N).astype(np.int64)

    # local node -> col' permutation pieces
    n_ = np.arange(CP)
    p_n, m_n = n_ // 16, n_ % 16
    colp_of_n = (m_n // 8) * 1024 + p_n * 8 + (m_n % 8)
    nodes_perm = np.empty(CP, dtype=np.int64)   # col' -> node
    nodes_perm[colp_of_n] = n_

    w1_h = np.ascontiguousarray(
        W1.reshape(KT, 128, F1).transpose(1, 0, 2).reshape(128, KT * F1)
    ).astype(NP_FP8)
    w2_h = np.ascontiguousarray(W2, dtype=np.float32)
    w3_h = np.ascontiguousarray(W3, dtype=np.float32)
    b1_h = np.ascontiguousarray(b1, dtype=np.float32).reshape(F1, 1)
    b2_h = np.ascontiguousarray(b2, dtype=np.float32).reshape(F2, 1)
    b3_h = np.ascontiguousarray(b3, dtype=np.float32).reshape(F3, 1)

    in_maps = []
    for d in range(NCORES):
        sl = slice(d * CP, (d + 1) * CP)
        dis_d = dis[sl]

        # ---- x^T fp8 tiled ----
        x8 = x[sl].astype(NP_FP8)              # [2048 nodes, 16384 feat]
        xt = x8[nodes_perm, :]                  # col' row order
        xt = xt.reshape(2, 1024, KT, 128).transpose(3, 0, 2, 1) \
               .reshape(128, 2 * KT * 1024)
        xt = np.ascontiguousarray(xt)

        # ---- packed adjacency ----
        m_ = (ut >= d * CP) & (ut < (d + 1) * CP)
        s_e = us[m_]
        t_e = ut[m_] - d * CP
        c_e = cnt[m_]
        ds_e, rem = s_e // CP, s_e % CP
        p_e, m_e = rem // 16, rem % 16
        hh_e = m_e // 8
        gg_e = (m_e % 8) // 4
        k_e = m_e % 4
        g_e = ds_e * 4 + hh_e * 2 + gg_e
        tp_e = colp_of_n[t_e]
        a_pk = np.zeros((128, 32, CP), dtype=np.uint8)
        np.bitwise_or.at(a_pk, (p_e, g_e, tp_e),
                         (1 << k_e).astype(np.uint8))

        # pre-expanded dense fp8 planes for the DMA-offloaded blocks
        adense = np.zeros((128, 4, 4, 2, CP), dtype=np.uint8)
        for (hs_b, d_b), s_b in ((( 0, 2), 0), ((0, 5), 1),
                                 ((1, 2), 2), ((1, 5), 3)):
            for k_b in range(4):
                for gg_b in range(2):
                    g_b = 4 * d_b + 2 * hs_b + gg_b
                    adense[:, s_b, k_b, gg_b, :] = \
                        ((a_pk[:, g_b, :] >> k_b) & 1) * 0x40

        # ---- residuals (count >= 2) ----
        rm = c_e >= 2
        rs, rt, rv = s_e[rm], tp_e[rm], (c_e[rm] - 1).astype(np.float32)
        nres = len(rs)
        assert nres <= 128, f"core {d}: {nres} residuals > 128"
        ridxa = np.full((128, 1), 100000, dtype=np.int32)
        ridxb = np.full((128, 1), 100000, dtype=np.int32)
        rcols = np.zeros((128, CP), dtype=np.float32)
        dsr, remr = rs // CP, rs % CP
        pr, mr = remr // 16, remr % 16
        hr, mmr = mr // 8, mr % 8
        rrow = dsr * 1024 + pr * 8 + mmr
        for i in range(nres):
            if hr[i] == 0:
                ridxa[i, 0] = rrow[i]
            else:
                ridxb[i, 0] = rrow[i]
            rcols[i, rt[i]] = 2.0 * rv[i]
        ridxa[nres:, 0] = 0   # padded slots read row 0, rcols row is 0
        rcols8 = rcols.astype(NP_FP8)

        # ---- scales ----
        disrep = np.broadcast_to(0.5 * dis_d[nodes_perm][None, :],
                                 (F1, CP)).astype(NP_BF16)

        in_maps.append({
            "xt_d": xt,
            "a_d": np.ascontiguousarray(a_pk.reshape(128, 32 * CP)),
            "adense_d": np.ascontiguousarray(
                adense.reshape(128, 4 * 8 * CP)),
            "w1_d": w1_h,
            "w2_d": w2_h,
            "w3_d": w3_h,
            "b1_d": b1_h,
            "b2_d": b2_h,
            "b3_d": b3_h,
            "dis16_d": np.ascontiguousarray(dis_d.reshape(128, MT)),
            "disrep_d": np.ascontiguousarray(disrep),
            "ridxa_d": ridxa,
            "ridxb_d": ridxb,
            "rcols_d": rcols8,
        })
    return in_maps


def _execute(in_maps, trace=False, trace_cores=None):
    nc = _get_program()
    return run_bass_kernel_spmd(nc, in_maps,
                                core_ids=list(range(NCORES)), trace=trace,
                                trace_cores=trace_cores)


def kernel(x, edge_index, W1, b1, W2, b2, W3, b3):
    in_maps = _preprocess(x, edge_index, W1, b1, W2, b2, W3, b3)
    res = _execute(in_maps, trace=False)
    return np.concatenate([r["out_d"] for r in res.results], axis=0)


# revision 38
# speedup vs baseline: 1.2000x; 1.2000x over previous
"""3-layer GCN on 8 Trainium2 NeuronCores — v3 (col-packed fp8).

Key ideas vs v2 (406us):
  - All matmuls drop DoubleRow and instead use tile_position column
    packing: out free dim F (64/32/16) < 128, so 2 (F=64) or 4 (F<=32)
    matmuls run CONCURRENTLY in disjoint 32/64-column groups of the PE
    array (measured 55.4 ns per [32,512] mm, 4.2x vs DR's 231 ns).
    Each PSUM bank [128,512] holds G group accumulators; a DVE add
    tree sums them in the epilogue.
  - Bit-plane expansion fused: one tensor_scalar per (hs, d, k) covers
    both gg groups and both target halves ([128, 2, 2048]B, 687 ns).
  - The per-source dis scale in wmul moves from DVE to the ACT engine
    (activation scale AP), freeing DVE for expansion.
  - Layers iterate hs-outer: srcA (hs=0) matmuls only need the first
    AllGather half, so they fill the AG-B latency window; L1 srcA
    blocks interleave into the back half of stage-1 h1.

Layout conventions (per core d; N=16384, CP=2048):
  local node n = p*16 + m,  m = h*8 + mm          (p partition, h half)
  column order in all [F, 2048] feature tiles:  col' = h*1024 + p*8 + mm
  src plane (d, h, mm): srcs {d*2048 + p*16 + h*8 + mm : p in 0..127}
  packed A byte [p, g, t'], g = d*4 + h*2 + gg, bit k:  plane mm = 4gg+k
  psf[li,h][p, d*8 + mm, f] = feature f of src (d, h, p, mm)
  AllGather row (per half): r = d*1024 + p*8 + mm.
"""

import numpy as np

import concourse.bass as bass
import concourse.mybir as mybir
import concourse.tile as tile
from concourse import bacc
from concourse.bass_utils import run_bass_kernel_spmd
from concourse.masks import make_identity

N = 16384
NCORES = 8
CP = N // NCORES          # 2048
F1, F2, F3 = 64, 32, 16
KT = N // 128             # 128 k tiles
MT = 16

F32 = mybir.dt.float32
FP8 = mybir.dt.float8e4
I32 = mybir.dt.int32
U8 = mybir.dt.uint8
BF16 = mybir.dt.bfloat16
NP_FP8 = mybir.dt.np(FP8)
NP_BF16 = mybir.dt.np(BF16)

_prog_cache = {}


def _build_program():
    nc = bacc.Bacc("TRN2", target_bir_lowering=False, debug=False,
                   num_devices=NCORES)

    # x^T fp8, pre-tiled: [p, (h, kt, t)] = x8[node n(h,t), kt*128+p]
    xt_d = nc.dram_tensor("xt_d", [128, 2 * KT * 1024], FP8,
                          kind="ExternalInput")
    # packed adjacency bit planes [p, (g, t')]
    a_d = nc.dram_tensor("a_d", [128, 32 * CP], U8, kind="ExternalInput")
    # pre-expanded dense fp8 planes for DENSE_BLOCKS (DMA-offloaded
    # expansion: rides the post-stage-1 idle DMA instead of the DVE)
    adense_d = nc.dram_tensor("adense_d", [128, 4 * 8 * CP], U8,
                              kind="ExternalInput")
    w1_d = nc.dram_tensor("w1_d", [128, KT * F1], FP8, kind="ExternalInput")
    w2_d = nc.dram_tensor("w2_d", [F1, F2], F32, kind="ExternalInput")
    w3_d = nc.dram_tensor("w3_d", [F2, F3], F32, kind="ExternalInput")
    b1_d = nc.dram_tensor("b1_d", [F1, 1], F32, kind="ExternalInput")
    b2_d = nc.dram_tensor("b2_d", [F2, 1], F32, kind="ExternalInput")
    b3_d = nc.dram_tensor("b3_d", [F3, 1], F32, kind="ExternalInput")
    dis16_d = nc.dram_tensor("dis16_d", [128, MT], F32, kind="ExternalInput")
    disrep_d = nc.dram_tensor("disrep_d", [F1, CP], BF16, kind="ExternalInput")
    ridxa_d = nc.dram_tensor("ridxa_d", [128, 1], I32, kind="ExternalInput")
    ridxb_d = nc.dram_tensor("ridxb_d", [128, 1], I32, kind="ExternalInput")
    rcols_d = nc.dram_tensor("rcols_d", [128, CP], FP8, kind="ExternalInput")
    out_d = nc.dram_tensor("out_d", [CP, F3], F32, kind="ExternalOutput")

    # AllGather bounce buffers per (layer, half)
    ps_in = {}
    ps_out = {}
    for li, F in ((1, F1), (2, F2), (3, F3)):
        for h in (0, 1):
            ps_in[li, h] = nc.dram_tensor(f"ps_in_{li}_{h}", [1024, F], FP8)
            ps_out[li, h] = nc.dram_tensor(f"ps_out_{li}_{h}", [8192, F],
                                           FP8, addr_space="Shared")

    with tile.TileContext(nc) as tc:
        with tc.tile_pool(name="const", bufs=1) as cpool, \
             tc.tile_pool(name="xt", bufs=6) as xtpool, \
             tc.tile_pool(name="exp", bufs=3) as epool, \
             tc.tile_pool(name="apk", bufs=4) as apool, \
             tc.tile_pool(name="work", bufs=2) as wpool, \
             tc.tile_pool(name="big", bufs=1) as bpool, \
             tc.tile_pool(name="psum", bufs=2, space="PSUM") as psum, \
             tc.tile_pool(name="psum_acc", bufs=1, space="PSUM") as psum_acc:

            # ---- constants -------------------------------------------------
            # w1 rides first on the scalar ring (needed at stage-1 kb0);
            # everything else goes on the (early-idle) gpsimd ring so the
            # x stream is never queued behind const traffic.
            w1_sb = cpool.tile([128, KT * F1], FP8, tag="w1")
            nc.scalar.dma_start(out=w1_sb[:], in_=w1_d[:, :])
            w2_sb = cpool.tile([F1, F2], F32, tag="w2")
            nc.gpsimd.dma_start(out=w2_sb[:], in_=w2_d[:, :])
            w3_sb = cpool.tile([F2, F3], F32, tag="w3")
            nc.gpsimd.dma_start(out=w3_sb[:], in_=w3_d[:, :])
            b_sb = {}
            for li, (bd, F) in ((1, (b1_d, F1)), (2, (b2_d, F2)),
                                (3, (b3_d, F3))):
                b_sb[li] = cpool.tile([F, 1], F32, tag=f"b{li}",
                                      name=f"b{li}_sb")
                nc.gpsimd.dma_start(out=b_sb[li][:], in_=bd[:, :])
            dis16_sb = cpool.tile([128, MT], F32, tag="dis16")
            nc.gpsimd.dma_start(out=dis16_sb[:], in_=dis16_d[:, :])
            disrep_sb = cpool.tile([F1, CP], BF16, tag="disrep")
            nc.gpsimd.dma_start(out=disrep_sb[:], in_=disrep_d[:, :])
            ridxa_sb = cpool.tile([128, 1], I32, tag="ridxa")
            nc.gpsimd.dma_start(out=ridxa_sb[:], in_=ridxa_d[:, :])
            ridxb_sb = cpool.tile([128, 1], I32, tag="ridxb")
            nc.gpsimd.dma_start(out=ridxb_sb[:], in_=ridxb_d[:, :])
            rcols_sb = cpool.tile([128, CP], FP8, tag="rcols")
            nc.gpsimd.dma_start(out=rcols_sb[:], in_=rcols_d[:, :])
            ident = cpool.tile([128, 128], F32, tag="ident")
            make_identity(nc, ident[:])

            # ---- PE warm-up: ~5us of back-to-back matmuls during the
            # initial DMA wait gets the HAM clock gate to K=8/8 before the
            # stage-1 stream arrives (cold PE runs at 1.2 instead of 2.4
            # GHz and stalls the whole pipeline behind it).  The tile uses
            # tag acc4, which the L1 aggregation reuses later.
            warm_ps = psum_acc.tile([128, 512], F32, tag="acc4", name="warm")

            def warm_mms(n):
                for _ in range(n):
                    nc.tensor.matmul(warm_ps[0:64, 0:128],
                                     lhsT=ident[:, 0:64],
                                     rhs=ident[:, :], start=True, stop=True,
                                     skip_group_check=True)
            warm_mms(48)

            # packed adjacency is STREAMED per expansion block (8MB/layer
            # rides the post-stage-1 idle DMA; keeps stage-1 pure-x and
            # frees 48KB of SBUF for deeper xt/et pools)

            # gathered features per (layer, half): [p, d*8+mm, F] fp8
            psf = {}
            for li, F in ((1, F1), (2, F2), (3, F3)):
                for h in (0, 1):
                    psf[li, h] = cpool.tile([128, 64, F], FP8,
                                            tag=f"psf{li}{h}",
                                            name=f"psf{li}{h}")

            def strided_m(t, m):
                """[F, CP] col'-ordered tile -> [F, 128] slice of nodes with
                n%16 == m (partitions p in order)."""
                return t[:].rearrange("f (hh pp mm) -> f hh mm pp",
                                      hh=2, mm=8)[:, m // 8, m % 8, :]

            # ---- stage 1 PSUM: per (h, c) one [128,512] bank, col groups
            # by kt parity (rows 0-63 even kt, 64-127 odd kt).
            s1_ps = [psum_acc.tile([128, 512], F32, tag=f"acc{c}",
                                   name=f"s1_ps{c}") for c in range(4)]

            # ---- aggregation helpers --------------------------------------
            # (DMA-offloaded pre-expanded blocks were tried here and were
            # a net loss: a 7us 2MB DMA per block bubbles the strictly
            # ordered et pool, vs 2.75us for DVE expansion.)
            DENSE_BLOCKS = {}

            def expand_block(li, hs, d):
                """expand the 8 planes of source block (hs, d) for ALL 2048
                targets -> et [128, k, gg, 2048] u8 (fp8 {0, 2.0})."""
                if (hs, d) in DENSE_BLOCKS:
                    s = DENSE_BLOCKS[hs, d]
                    et = epool.tile([128, 4, 2, 2048], U8, tag="exp",
                                    name=f"e{li}{hs}{d}")
                    ring = nc.sync if s % 2 == 0 else nc.scalar
                    ring.dma_start(
                        out=et[:],
                        in_=adense_d[:, s * 8 * CP:(s + 1) * 8 * CP]
                            .rearrange("p (k g t) -> p k g t", k=4, g=2))
                    return et
                g0 = 4 * d + 2 * hs
                asl = apool.tile([128, 2, 2048], U8, tag="apack",
                                 name=f"a{li}{hs}{d}")
                aring = nc.sync if (hs * 8 + d) % 2 == 0 else nc.scalar
                aring.dma_start(
                    out=asl[:],
                    in_=a_d[:, g0 * CP:(g0 + 2) * CP]
                        .rearrange("p (g t) -> p g t", g=2))
                et = epool.tile([128, 4, 2, 2048], U8, tag="exp",
                                name=f"e{li}{hs}{d}")
                # Mild deprioritization (about one layer's instruction
                # span): keeps each layer's pool-paced expansions BEHIND
                # that layer's latency-critical epilogue ops in the DVE
                # order, without reordering across layers (a global
                # deprioritization was tried and created worse couplings).
                with tc.high_priority(offset=-1500):
                    for k in range(4):
                        nc.vector.tensor_scalar(
                            out=et[:, k, :, :].bitcast(I32),
                            in0=asl[:].bitcast(I32),
                            scalar1=0x01010101 << k, scalar2=6 - k,
                            op0=mybir.AluOpType.bitwise_and,
                            op1=mybir.AluOpType.logical_shift_left)
                return et

            def agg_block_mms(li, F, G, hs, d, banks, started, last):
                """col-packed matmuls consuming one expanded block.  F<=32
                packs 4-way by plane k; F==64 feature-splits each plane so
                it also packs 4-way (cold-clock robust)."""
                et = expand_block(li, hs, d)
                fsplit = 2 if F == 64 else 1
                Fs = F // fsplit
                for gg in range(2):
                    for cc in range(4):
                        for k in range(4):
                            lhsT = psf[li, hs][:, d * 8 + 4 * gg + k, :]
                            rhs = et[:, k, gg, cc * 512:(cc + 1) * 512] \
                                .bitcast(FP8)
                            for fh in range(fsplit):
                                if fsplit == 2:
                                    g = 2 * (k % 2) + fh
                                    lh = lhsT[:, 32 * fh:32 * fh + 32]
                                    # groups 0,1 (k even, rows 0-63) stay
                                    # open for the resid matmuls
                                    stp = (last and gg == 1 and k == 3)
                                else:
                                    g = k
                                    lh = lhsT
                                    stp = (last and gg == 1 and k > 0)
                                cb = 32 * g
                                key = (cc, cb)
                                st = key not in started
                                started.add(key)
                                nc.tensor.matmul(
                                    banks[cc][cb:cb + Fs, :], lhsT=lh,
                                    rhs=rhs, start=st, stop=stp,
                                    tile_position=(0, cb))

            def resid_gather(li, F):
                rh = wpool.tile([128, F], FP8, tag="residh",
                                name=f"residh{li}")
                for h, ridx in ((0, ridxa_sb), (1, ridxb_sb)):
                    nc.gpsimd.indirect_dma_start(
                        out=rh[:], out_offset=None,
                        in_=ps_out[li, h].ap(),
                        in_offset=bass.IndirectOffsetOnAxis(
                            ap=ridx[:, :1], axis=0),
                        bounds_check=8191, oob_is_err=False)
                return rh

            def resid_mms(rh, F, banks):
                for cc in range(4):
                    nc.tensor.matmul(
                        banks[cc][0:F, :], lhsT=rh[:],
                        rhs=rcols_sb[:, cc * 512:(cc + 1) * 512],
                        start=False, stop=True, skip_group_check=True,
                        tile_position=(0, 0))

            def psum_group_sum(bank, F, G, zs, tagp):
                """sum the G col-group accumulators of one PSUM bank into
                zs [F, 512] (SBUF).  ACT copies the upper groups out of
                PSUM (cross-partition), DVE adds."""
                gbase = (64,) if G == 2 else (32, 64, 96)
                cp = []
                for gi, gb in enumerate(gbase):
                    c = wpool.tile([F, 512], F32, tag=f"cp{gi}",
                                   name=f"cp{tagp}_{gi}")
                    nc.scalar.activation(
                        c[:], bank[gb:gb + F, :],
                        mybir.ActivationFunctionType.Identity,
                        bias=0.0)
                    cp.append(c)
                if G == 2:
                    nc.vector.tensor_tensor(zs[:], bank[0:F, :], cp[0][:],
                                            mybir.AluOpType.add)
                else:
                    t1 = wpool.tile([F, 512], F32, tag="t1",
                                    name=f"t1{tagp}")
                    nc.vector.tensor_tensor(t1[:], bank[0:F, :], cp[0][:],
                                            mybir.AluOpType.add)
                    t2 = wpool.tile([F, 512], F32, tag="t2",
                                    name=f"t2{tagp}")
                    nc.vector.tensor_tensor(t2[:], cp[1][:], cp[2][:],
                                            mybir.AluOpType.add)
                    nc.vector.tensor_tensor(zs[:], t1[:], t2[:],
                                            mybir.AluOpType.add)

            def agg_epilogue(li, ht, F, G, banks, hT_sb):
                func = (mybir.ActivationFunctionType.Relu if li < 3
                        else mybir.ActivationFunctionType.Identity)
                for cc in (2 * ht, 2 * ht + 1):
                    sl = slice(cc * 512, (cc + 1) * 512)
                    zs = wpool.tile([F, 512], F32, tag="zs",
                                    name=f"zs{li}_{cc}")
                    psum_group_sum(banks[cc], F, G, zs, f"{li}_{cc}")
                    zt = wpool.tile([F, 512], F32, tag="zt",
                                    name=f"zt{li}_{cc}")
                    nc.vector.tensor_tensor(zt[:], zs[:],
                                            disrep_sb[:F, sl],
                                            mybir.AluOpType.mult)
                    nc.scalar.activation(hT_sb[:, sl], zt[:], func,
                                         bias=b_sb[li][:, 0:1])

            def wmul_gather(hT_sb, li, F_nxt, w_sb, h):
                """W-mul (or stage-1 transpose) + dis pre-scale + fp8,
                bounce to DRAM, AllGather half h, reload psf[li, h].
                All 8 m-slices land in ONE PSUM bank so a single DVE
                multiply (broadcast dis16) replaces 8 serial ACT ops."""
                pt = psum.tile([128, 8, F_nxt], F32, tag="wmul",
                               name=f"wm{li}_{h}")
                for mm in range(8):
                    m = h * 8 + mm
                    if w_sb is None:
                        nc.tensor.transpose(pt[:, mm, :],
                                            strided_m(hT_sb, m),
                                            ident[:F_nxt, :F_nxt])
                    else:
                        nc.tensor.matmul(pt[:, mm, :],
                                         lhsT=strided_m(hT_sb, m),
                                         rhs=w_sb[:], start=True, stop=True)
                ps_local = wpool.tile([128, 8, F_nxt], FP8,
                                      tag=f"psl{li}", name=f"psl{li}_{h}")
                d16b = dis16_sb[:, h * 8:h * 8 + 8] \
                    .rearrange("p (m o) -> p m o", o=1) \
                    .broadcast_to((128, 8, F_nxt))
                nc.vector.tensor_tensor(ps_local[:], pt[:], d16b,
                                        mybir.AluOpType.mult)
                # bounce rides gpsimd: on sync it would stall queued x
                # tiles behind its wait for ps_local
                nc.gpsimd.dma_start(
                    out=ps_in[li, h].ap().rearrange("(p mm) f -> p mm f",
                                                    p=128),
                    in_=ps_local[:])
                nc.gpsimd.collective_compute(
                    "AllGather", mybir.AluOpType.bypass,
                    replica_groups=[list(range(NCORES))],
                    ins=[ps_in[li, h].ap().opt()],
                    outs=[ps_out[li, h].ap().opt()],
                )
                nc.scalar.dma_start(
                    out=psf[li, h][:].rearrange("p (d mm) f -> p d mm f",
                                                d=8),
                    in_=ps_out[li, h].ap().rearrange("(d p mm) f -> p d mm f",
                                                     d=8, p=128))

            # ---- stage 1 ---------------------------------------------------
            s1rings = [nc.sync, nc.scalar]

            def stage1_half(h, extra_pe=None):
                xt_tile = None
                for kb in range(KT // 8):
                    xt_tile = xtpool.tile([128, 8, 1024], FP8, tag="xt",
                                          name=f"xt{h}_{kb}")
                    off = (h * KT + kb * 8) * 1024
                    s1rings[kb % 2].dma_start(
                        out=xt_tile[:],
                        in_=xt_d[:, off:off + 8 * 1024]
                            .rearrange("p (b t) -> p b t", b=8))
                    if h == 0 and kb < 6:
                        warm_mms(6)  # keep HAM at K=8/8 through DMA ramp
                    if extra_pe is not None and kb in extra_pe:
                        extra_pe[kb]()
                    for b in range(8):
                        kt = kb * 8 + b
                        kp = kt % 2
                        for c in range(2):
                            for fh in range(2):
                                g = 2 * kp + fh
                                nc.tensor.matmul(
                                    s1_ps[2 * h + c][32 * g:32 * g + 32, :],
                                    lhsT=w1_sb[:, kt * F1 + 32 * fh:
                                               kt * F1 + 32 * fh + 32],
                                    rhs=xt_tile[:, b, c * 512:(c + 1) * 512],
                                    start=(kt < 2), stop=(kt >= KT - 2),
                                    tile_position=(0, 32 * g))
                return xt_tile

            def stage1_epilogue(h, p1t_sb):
                for c in range(2):
                    sl = slice(h * 1024 + c * 512, h * 1024 + (c + 1) * 512)
                    bank = s1_ps[2 * h + c]
                    cp = wpool.tile([64, 512], F32, tag="cp0",
                                    name=f"s1cp_{h}_{c}")
                    nc.scalar.activation(
                        cp[:], bank[64:128, :],
                        mybir.ActivationFunctionType.Identity, bias=0.0)
                    nc.vector.tensor_tensor(p1t_sb[:, sl], bank[0:64, :],
                                            cp[:], mybir.AluOpType.add)
                wmul_gather(p1t_sb, 1, F1, None, h)

            # ======== stage 1 half 0 ========
            stage1_half(0)
            p1t_sb = bpool.tile([F1, CP], F32, tag="hT1")
            stage1_epilogue(0, p1t_sb)

            # L1 aggregation state
            agg1_ps = [psum_acc.tile([128, 512], F32,
                                     tag=f"acc{c if c < 2 else c + 2}",
                                     name=f"agg1_{c}")
                       for c in range(4)]
            started1 = set()

            # NOTE: no L1 blocks interleave into stage-1 h1.  NEFF launch
            # skew across cores is ~50us, so AG-A completes AFTER the
            # (DMA-bound) stage-1 stream ends; a psf-waiting matmul placed
            # inside the stage-1 PE stream would stall its tail.

            # ======== stage 1 half 1 ========
            last_xt = stage1_half(1)
            warm_mms(16)
            stage1_epilogue(1, p1t_sb)
            # PE filler across the AG-A latency window: dependency-free
            # matmuls on the last x tile keep the HAM clock gate at 8/8
            # (a >3us PE idle here re-throttles the PE to 1.2 GHz and it
            # has been observed to stay cold for ~50us afterwards).
            for i in range(64):
                nc.tensor.matmul(warm_ps[0:32, :],
                                 lhsT=w1_sb[:, 32 * (i % 8):32 * (i % 8) + 32],
                                 rhs=last_xt[:, 7, 0:512], start=True,
                                 stop=True, skip_group_check=True)

            # ======== layers ========
            specs = [(1, F1, 2, F2, w2_sb), (2, F2, 4, F3, w3_sb),
                     (3, F3, 4, None, None)]
            for li, F, G, F_nxt, w_sb in specs:
                if li == 1:
                    agg_ps, started = agg1_ps, started1
                else:
                    agg_ps = [psum_acc.tile([128, 512], F32,
                                            tag=f"acc{c if c < 2 else c + 2}",
                                            name=f"agg{li}_{c}")
                              for c in range(4)]
                    started = set()
                rh = resid_gather(li, F)
                hT_sb = bpool.tile([F, CP], F32,
                                   tag="hT1" if li == 1 else "hTb",
                                   name=f"hT{li}")
                for hs in (0, 1):
                    for d in range(8):
                        agg_block_mms(li, F, G, hs, d, agg_ps, started,
                                      last=(hs == 1 and d == 7))
                resid_mms(rh, F, agg_ps)
                for ht in (0, 1):
                    agg_epilogue(li, ht, F, G, agg_ps, hT_sb)
                    if li < 3:
                        wmul_gather(hT_sb, li + 1, F_nxt, w_sb, ht)
                    else:
                        h3 = wpool.tile([128, 8, F3], F32, tag="h3",
                                        name=f"h3_{ht}")
                        for mm in range(8):
                            m = ht * 8 + mm
                            pt = psum.tile([128, F3], F32, tag="wmul",
                                           name=f"tr3_{ht}_{mm}")
                            nc.tensor.transpose(pt[:], strided_m(hT_sb, m),
                                                ident[:F3, :F3])
                            nc.vector.tensor_copy(h3[:, mm, :], pt[:])
                        mx = wpool.tile([128, 8], F32, tag="mx",
                                        name=f"mx{ht}")
                        nc.vector.reduce_max(mx[:], h3[:],
                                             mybir.AxisListType.X,
                                             negate=True)
                        mxb = mx[:].rearrange("p (m o) -> p m o", o=1) \
                                   .broadcast_to((128, 8, F3))
                        ex = wpool.tile([128, 8, F3], F32, tag="ex",
                                        name=f"ex{ht}")
                        nc.vector.tensor_tensor(ex[:], h3[:], mxb,
                                                mybir.AluOpType.add)
                        nc.scalar.activation(ex[:], ex[:],
                                             mybir.ActivationFunctionType.Exp)
                        sm = wpool.tile([128, 8], F32, tag="sm",
                                        name=f"sm{ht}")
                        nc.vector.reduce_sum(sm[:], ex[:],
                                             mybir.AxisListType.X)
                        rc = wpool.tile([128, 8], F32, tag="rc",
                                        name=f"rc{ht}")
                        nc.vector.reciprocal(rc[:], sm[:])
                        rcb = rc[:].rearrange("p (m o) -> p m o", o=1) \
                                   .broadcast_to((128, 8, F3))
                        o_sb = wpool.tile([128, 8, F3], F32, tag="osm",
                                          name=f"osm{ht}")
                        nc.vector.tensor_tensor(o_sb[:], ex[:], rcb,
                                                mybir.AluOpType.mult)
                        nc.sync.dma_start(
                            out=out_d.ap().rearrange(
                                "(p hh mm) c -> p hh mm c",
                                p=128, hh=2)[:, ht, :, :],
                            in_=o_sb[:])

    nc.compile()
    return nc


def _get_program():
    if "nc" not in _prog_cache:
        _prog_cache["nc"] = _build_program()
    return _prog_cache["nc"]


def _preprocess(x, edge_index, W1, b1, W2, b2, W3, b3):
    x = np.asarray(x, dtype=np.float32)
    ei = np.asarray(edge_index)
    row = ei[0].astype(np.int64)
    col = ei[1].astype(np.int64)

    deg = np.bincount(col, minlength=N).astype(np.float32) + 1.0
    dis = (1.0 / np.sqrt(deg)).astype(np.float32)

    # unique edge cells incl. self loops
    pair = np.concatenate([row * N + col,
                           np.arange(N, dtype=np.int64) * (N + 1)])
    u, cnt = np.unique(pair, return_counts=True)
    us = (u // N).astype(np.int64)
    ut = (u % N).astype(np.int64)

    # local node -> col' permutation pieces
    n_ = np.arange(CP)
    p_n, m_n = n_ // 16, n_ % 16
    colp_of_n = (m_n // 8) * 1024 + p_n * 8 + (m_n % 8)
    nodes_perm = np.empty(CP, dtype=np.int64)   # col' -> node
    nodes_perm[colp_of_n] = n_

    w1_h = np.ascontiguousarray(
        W1.reshape(KT, 128, F1).transpose(1, 0, 2).reshape(128, KT * F1)
    ).astype(NP_FP8)
    w2_h = np.ascontiguousarray(W2, dtype=np.float32)
    w3_h = np.ascontiguousarray(W3, dtype=np.float32)
    b1_h = np.ascontiguousarray(b1, dtype=np.float32).reshape(F1, 1)
    b2_h = np.ascontiguousarray(b2, dtype=np.float32).reshape(F2, 1)
    b3_h = np.ascontiguousarray(b3, dtype=np.float32).reshape(F3, 1)

    in_maps = []
    for d in range(NCORES):
        sl = slice(d * CP, (d + 1) * CP)
        dis_d = dis[sl]

        # ---- x^T fp8 tiled ----
        x8 = x[sl].astype(NP_FP8)              # [2048 nodes, 16384 feat]
        xt = x8[nodes_perm, :]                  # col' row order
        xt = xt.reshape(2, 1024, KT, 128).transpose(3, 0, 2, 1) \
               .reshape(128, 2 * KT * 1024)
        xt = np.ascontiguousarray(xt)

        # ---- packed adjacency ----
        m_ = (ut >= d * CP) & (ut < (d + 1) * CP)
        s_e = us[m_]
        t_e = ut[m_] - d * CP
        c_e = cnt[m_]
        ds_e, rem = s_e // CP, s_e % CP
        p_e, m_e = rem // 16, rem % 16
        hh_e = m_e // 8
        gg_e = (m_e % 8) // 4
        k_e = m_e % 4
        g_e = ds_e * 4 + hh_e * 2 + gg_e
        tp_e = colp_of_n[t_e]
        a_pk = np.zeros((128, 32, CP), dtype=np.uint8)
        np.bitwise_or.at(a_pk, (p_e, g_e, tp_e),
                         (1 << k_e).astype(np.uint8))

        # pre-expanded dense fp8 planes for the DMA-offloaded blocks
        adense = np.zeros((128, 4, 4, 2, CP), dtype=np.uint8)
        for (hs_b, d_b), s_b in ((( 0, 2), 0), ((0, 5), 1),
                                 ((1, 2), 2), ((1, 5), 3)):
            for k_b in range(4):
                for gg_b in range(2):
                    g_b = 4 * d_b + 2 * hs_b + gg_b
                    adense[:, s_b, k_b, gg_b, :] = \
                        ((a_pk[:, g_b, :] >> k_b) & 1) * 0x40

        # ---- residuals (count >= 2) ----
        rm = c_e >= 2
        rs, rt, rv = s_e[rm], tp_e[rm], (c_e[rm] - 1).astype(np.float32)
        nres = len(rs)
        assert nres <= 128, f"core {d}: {nres} residuals > 128"
        ridxa = np.full((128, 1), 100000, dtype=np.int32)
        ridxb = np.full((128, 1), 100000, dtype=np.int32)
        rcols = np.zeros((128, CP), dtype=np.float32)
        dsr, remr = rs // CP, rs % CP
        pr, mr = remr // 16, remr % 16
        hr, mmr = mr // 8, mr % 8
        rrow = dsr * 1024 + pr * 8 + mmr
        for i in range(nres):
            if hr[i] == 0:
                ridxa[i, 0] = rrow[i]
            else:
                ridxb[i, 0] = rrow[i]
            rcols[i, rt[i]] = 2.0 * rv[i]
        ridxa[nres:, 0] = 0   # padded slots read row 0, rcols row is 0
        rcols8 = rcols.astype(NP_FP8)

        # ---- scales ----
        disrep = np.broadcast_to(0.5 * dis_d[nodes_perm][None, :],
                                 (F1, CP)).astype(NP_BF16)

        in_maps.append({
            "xt_d": xt,
            "a_d": np.ascontiguousarray(a_pk.reshape(128, 32 * CP)),
            "adense_d": np.ascontiguousarray(
                adense.reshape(128, 4 * 8 * CP)),
            "w1_d": w1_h,
            "w2_d": w2_h,
            "w3_d": w3_h,
            "b1_d": b1_h,
            "b2_d": b2_h,
            "b3_d": b3_h,
            "dis16_d": np.ascontiguousarray(dis_d.reshape(128, MT)),
            "disrep_d": np.ascontiguousarray(disrep),
            "ridxa_d": ridxa,
            "ridxb_d": ridxb,
            "rcols_d": rcols8,
        })
    return in_maps


def _execute(in_maps, trace=False, trace_cores=None):
    nc = _get_program()
    return run_bass_kernel_spmd(nc, in_maps,
                                core_ids=list(range(NCORES)), trace=trace,
                                trace_cores=trace_cores)


def kernel(x, edge_index, W1, b1, W2, b2, W3, b3):
    in_maps = _preprocess(x, edge_index, W1, b1, W2, b2, W3, b3)
    res = _execute(in_maps, trace=False)
    return np.concatenate([r["out_d"] for r in res.results], axis=0)
